# revision 1
# baseline (speedup 1.0000x reference)
"""2-layer GCN (GCNConv -> ReLU -> GCNConv) on 8 TRN2 NeuronCores.

Strategy (sliced-ELLPACK, node sharding):
  GCN algebra: out = D^-1/2 (A+I) D^-1/2 (relu(D^-1/2 (A+I) D^-1/2 x W1 + b1)) W2 + b2.
  The normalization is separable (norm_e = dinv[row]*dinv[col]) and aggregation
  is linear, so each layer is: per-node scale -> unweighted neighbor-sum ->
  per-node scale -> dense matmul. Self-loop contributions are dense adds.

  Host (pure index layout, no model math): sort nodes by in-degree, pack each
  node's incoming edges into a padded row of slots (sliced ELLPACK, slices of
  128 nodes, groups of 32 slices sharing a width). Per-slot it gathers the
  *input* features x[row] and the integer structural count deg[row]. Device
  computes all floating point model math: rsqrt, scalings, segmented
  reductions (DVE tensor_reduce), both layer matmuls, relu.

  Two launches: A computes layer 1 + the per-node scalar sigma = (dinv*h)@W2;
  host re-shards sigma into the same slot layout (index gather only);
  B reduces sigma-slots and finishes layer 2. Nodes are sharded across the 8
  cores round-robin by 128-node slice; edge slots live with their target node.
"""
import numpy as np

P = 128
N_CORES = 8
NSL = 256           # local slices per core  -> 2048 global slices
NGRP = 8            # groups of 32 slices sharing one slot width
NRANKS = 2048 * P   # padded rank space (262144 >= 250000)

TRACE = False
_cache = {}


def _install_ntff_shim():
    import contextlib, ctypes, sys, types
    if "antenv.axon_hooks" in sys.modules:
        return
    try:
        lib = ctypes.CDLL("/opt/axon/libaxon_pjrt.so")
        if not hasattr(lib, "axon_start_nrt_profile"):
            return
        lib.axon_start_nrt_profile.argtypes = [ctypes.POINTER(ctypes.c_int64), ctypes.c_size_t]
        lib.axon_start_nrt_profile.restype = ctypes.c_int64
        lib.axon_stop_nrt_profile.argtypes = [ctypes.c_char_p]
        lib.axon_stop_nrt_profile.restype = ctypes.c_int64
    except OSError:
        return

    @contextlib.contextmanager
    def _hook(output_dir, device_ids):
        import jax
        jax.devices()
        if device_ids:
            ids = (ctypes.c_int64 * len(device_ids))(*device_ids)
            rc = lib.axon_start_nrt_profile(ids, len(device_ids))
        else:
            rc = lib.axon_start_nrt_profile(None, 0)
        if rc != 0:
            raise RuntimeError(f"axon_start_nrt_profile rc={rc}")
        try:
            yield
        finally:
            lib.axon_stop_nrt_profile(str(output_dir).encode())

    mod = types.ModuleType("antenv.axon_hooks")
    mod.get_axon_ntff_profile_hook = lambda: _hook
    mod.set_axon_ntff_profile_hook = lambda h: None
    sys.modules["antenv.axon_hooks"] = mod


def _build_programs(TOT, W_grp, off):
    import concourse.bass as bass
    import concourse.bacc as bacc
    import concourse.tile as tile
    import concourse.mybir as mybir

    f32 = mybir.dt.float32
    AF = mybir.ActivationFunctionType
    ALU = mybir.AluOpType

    def reduce_groups(nc, tc, pool, slots_dram, agg_tile):
        """agg_tile[:, g*32:(g+1)*32] = segmented sums of the slot array."""
        for g in range(NGRP):
            Wg = int(W_grp[g])
            if Wg == 0:
                continue
            t = pool.tile([P, 32 * Wg], f32, tag="slots")
            nc.sync.dma_start(t[:], slots_dram.ap()[:, int(off[g]):int(off[g]) + 32 * Wg])
            yield g, Wg, t

    # ---------------- program A ----------------
    ncA = bacc.Bacc("TRN2", target_bir_lowering=False, debug=False, num_devices=N_CORES)
    x0s = ncA.dram_tensor("x0s", [P, TOT], f32, kind="ExternalInput")
    x1s = ncA.dram_tensor("x1s", [P, TOT], f32, kind="ExternalInput")
    dgs = ncA.dram_tensor("dgs", [P, TOT], f32, kind="ExternalInput")
    x0t = ncA.dram_tensor("x0t", [P, NSL], f32, kind="ExternalInput")
    x1t = ncA.dram_tensor("x1t", [P, NSL], f32, kind="ExternalInput")
    dgt = ncA.dram_tensor("dgt", [P, NSL], f32, kind="ExternalInput")
    wb = ncA.dram_tensor("wb", [P, 65], f32, kind="ExternalInput")
    sOut = ncA.dram_tensor("sOut", [P, NSL], f32, kind="ExternalOutput")

    with tile.TileContext(ncA) as tc:
        with tc.tile_pool(name="slots", bufs=4) as pool, \
             tc.tile_pool(name="persist", bufs=1) as pp:
            agg0 = pp.tile([P, NSL], f32)
            agg1 = pp.tile([P, NSL], f32)
            xt0 = pp.tile([P, NSL], f32)
            ncA.sync.dma_start(xt0[:], x0t.ap())
            xt1 = pp.tile([P, NSL], f32)
            ncA.sync.dma_start(xt1[:], x1t.ap())
            dt = pp.tile([P, NSL], f32)
            ncA.sync.dma_start(dt[:], dgt.ap())
            wbt = pp.tile([P, 65], f32)
            ncA.sync.dma_start(wbt[:], wb.ap())
            dinv = pp.tile([P, NSL], f32)
            ncA.scalar.activation(dinv[:], dt[:], AF.Sqrt)
            ncA.vector.reciprocal(dinv[:], dinv[:])
            for g in range(NGRP):
                Wg = int(W_grp[g])
                o = int(off[g])
                if Wg == 0:
                    ncA.gpsimd.memset(agg0[:, g * 32:(g + 1) * 32], 0.0)
                    ncA.gpsimd.memset(agg1[:, g * 32:(g + 1) * 32], 0.0)
                    continue
                td = pool.tile([P, 32 * Wg], f32, tag="dg")
                ncA.sync.dma_start(td[:], dgs.ap()[:, o:o + 32 * Wg])
                t0 = pool.tile([P, 32 * Wg], f32, tag="x0")
                ncA.sync.dma_start(t0[:], x0s.ap()[:, o:o + 32 * Wg])
                t1 = pool.tile([P, 32 * Wg], f32, tag="x1")
                ncA.sync.dma_start(t1[:], x1s.ap()[:, o:o + 32 * Wg])
                r = pool.tile([P, 32 * Wg], f32, tag="r")
                ncA.scalar.activation(r[:], td[:], AF.Sqrt)
                ncA.vector.reciprocal(r[:], r[:])
                ncA.vector.tensor_tensor(out=t0[:], in0=t0[:], in1=r[:], op=ALU.mult)
                ncA.vector.tensor_tensor(out=t1[:], in0=t1[:], in1=r[:], op=ALU.mult)
                ncA.vector.tensor_reduce(
                    out=agg0[:, g * 32:(g + 1) * 32],
                    in_=t0[:].rearrange("p (n w) -> p n w", w=Wg),
                    axis=mybir.AxisListType.X, op=ALU.add)
                ncA.vector.tensor_reduce(
                    out=agg1[:, g * 32:(g + 1) * 32],
                    in_=t1[:].rearrange("p (n w) -> p n w", w=Wg),
                    axis=mybir.AxisListType.X, op=ALU.add)

            # z_f = dinv * (agg_f + dinv * x_f)
            tmp = pp.tile([P, NSL], f32)
            z0 = pp.tile([P, NSL], f32)
            z1 = pp.tile([P, NSL], f32)
            ncA.vector.tensor_tensor(out=tmp[:], in0=dinv[:], in1=xt0[:], op=ALU.mult)
            ncA.vector.tensor_tensor(out=tmp[:], in0=tmp[:], in1=agg0[:], op=ALU.add)
            ncA.vector.tensor_tensor(out=z0[:], in0=tmp[:], in1=dinv[:], op=ALU.mult)
            ncA.vector.tensor_tensor(out=tmp[:], in0=dinv[:], in1=xt1[:], op=ALU.mult)
            ncA.vector.tensor_tensor(out=tmp[:], in0=tmp[:], in1=agg1[:], op=ALU.add)
            ncA.vector.tensor_tensor(out=z1[:], in0=tmp[:], in1=dinv[:], op=ALU.mult)

            # h_j = relu(z0*W1[0,j] + z1*W1[1,j] + b1[j]); acc = sum_j h_j*W2[j]
            acc = pp.tile([P, NSL], f32)
            hj = pp.tile([P, NSL], f32)
            tmp2 = pp.tile([P, NSL], f32)
            for j in range(16):
                ncA.vector.tensor_tensor(
                    out=hj[:], in0=z0[:],
                    in1=wbt[:, j:j + 1].to_broadcast([P, NSL]), op=ALU.mult)
                ncA.vector.tensor_tensor(
                    out=tmp2[:], in0=z1[:],
                    in1=wbt[:, 16 + j:17 + j].to_broadcast([P, NSL]), op=ALU.mult)
                ncA.vector.tensor_tensor(out=hj[:], in0=hj[:], in1=tmp2[:], op=ALU.add)
                ncA.scalar.activation(hj[:], hj[:], AF.Relu,
                                      bias=wbt[:, 32 + j:33 + j])
                ncA.vector.tensor_tensor(
                    out=hj[:], in0=hj[:],
                    in1=wbt[:, 48 + j:49 + j].to_broadcast([P, NSL]), op=ALU.mult)
                if j == 0:
                    ncA.vector.tensor_copy(acc[:], hj[:])
                else:
                    ncA.vector.tensor_tensor(out=acc[:], in0=acc[:], in1=hj[:], op=ALU.add)
            # sigma = dinv * acc
            ncA.vector.tensor_tensor(out=acc[:], in0=acc[:], in1=dinv[:], op=ALU.mult)
            ncA.sync.dma_start(sOut.ap(), acc[:])
    ncA.compile()

    # ---------------- program B ----------------
    ncB = bacc.Bacc("TRN2", target_bir_lowering=False, debug=False, num_devices=N_CORES)
    sgs = ncB.dram_tensor("sgs", [P, TOT], f32, kind="ExternalInput")
    dgtB = ncB.dram_tensor("dgtB", [P, NSL], f32, kind="ExternalInput")
    stB = ncB.dram_tensor("stB", [P, NSL], f32, kind="ExternalInput")
    wbB = ncB.dram_tensor("wbB", [P, 65], f32, kind="ExternalInput")
    out = ncB.dram_tensor("out", [P, NSL], f32, kind="ExternalOutput")

    with tile.TileContext(ncB) as tc:
        with tc.tile_pool(name="slots", bufs=3) as pool, \
             tc.tile_pool(name="persist", bufs=1) as pp:
            agg = pp.tile([P, NSL], f32)
            for g in range(NGRP):
                Wg = int(W_grp[g])
                o = int(off[g])
                if Wg == 0:
                    ncB.gpsimd.memset(agg[:, g * 32:(g + 1) * 32], 0.0)
                    continue
                t = pool.tile([P, 32 * Wg], f32, tag="s")
                ncB.sync.dma_start(t[:], sgs.ap()[:, o:o + 32 * Wg])
                ncB.vector.tensor_reduce(
                    out=agg[:, g * 32:(g + 1) * 32],
                    in_=t[:].rearrange("p (n w) -> p n w", w=Wg),
                    axis=mybir.AxisListType.X, op=ALU.add)
            dt = pp.tile([P, NSL], f32)
            ncB.sync.dma_start(dt[:], dgtB.ap())
            st = pp.tile([P, NSL], f32)
            ncB.sync.dma_start(st[:], stB.ap())
            wbt = pp.tile([P, 65], f32)
            ncB.sync.dma_start(wbt[:], wbB.ap())
            dinv = pp.tile([P, NSL], f32)
            ncB.scalar.activation(dinv[:], dt[:], AF.Sqrt)
            ncB.vector.reciprocal(dinv[:], dinv[:])
            o1 = pp.tile([P, NSL], f32)
            ncB.vector.tensor_tensor(out=o1[:], in0=agg[:], in1=st[:], op=ALU.add)
            ncB.vector.tensor_tensor(out=o1[:], in0=o1[:], in1=dinv[:], op=ALU.mult)
            ncB.vector.tensor_tensor(
                out=o1[:], in0=o1[:],
                in1=wbt[:, 64:65].to_broadcast([P, NSL]), op=ALU.add)
            ncB.sync.dma_start(out.ap(), o1[:])
    ncB.compile()
    return ncA, ncB


def kernel(x, edge_index, W1, b1, W2, b2, n_nodes):
    from concourse.bass_utils import run_bass_kernel_spmd

    N = int(n_nodes)
    x = np.asarray(x, dtype=np.float32)
    ei = np.asarray(edge_index)
    row = ei[0].astype(np.int64)
    col = ei[1].astype(np.int64)
    W1 = np.asarray(W1, np.float32); b1 = np.asarray(b1, np.float32)
    W2 = np.asarray(W2, np.float32); b2 = np.asarray(b2, np.float32)
    E = row.shape[0]

    # ---- host index layout (structural only) ----
    deg = np.bincount(col, minlength=N) + 1           # includes self-loop
    indeg = deg - 1
    order = np.argsort(-deg, kind="stable")           # rank -> node
    rank_of = np.empty(N, np.int64)
    rank_of[order] = np.arange(N)

    indeg_byrank = np.zeros(NRANKS, np.int64)
    indeg_byrank[:N] = indeg[order]
    W_slice = indeg_byrank.reshape(2048, P).max(axis=1)          # per global slice
    W_by_l = W_slice.reshape(NSL, N_CORES).max(axis=1)           # max over cores
    W_grp = W_by_l.reshape(NGRP, 32).max(axis=1)                 # per group
    off = np.zeros(NGRP, np.int64)
    np.cumsum(32 * W_grp[:-1], out=off[1:])
    TOT = int(off[-1] + 32 * W_grp[-1])

    key = (TOT, tuple(W_grp.tolist()))
    if key not in _cache:
        if TRACE:
            _install_ntff_shim()
        _cache[key] = _build_programs(TOT, W_grp, off)
    ncA, ncB = _cache[key]

    # ---- per-edge slot placement ----
    re = rank_of[col]
    sl = re >> 7
    pe = re & 127
    ce = sl % N_CORES
    le = sl // N_CORES
    ge = le >> 5
    sidx = np.argsort(re, kind="stable")
    re_s = re[sidx]
    runstart = np.empty(E, bool)
    runstart[0] = True
    np.not_equal(re_s[1:], re_s[:-1], out=runstart[1:])
    starts = np.flatnonzero(runstart)
    rid = np.cumsum(runstart) - 1
    slot = np.empty(E, np.int64)
    slot[sidx] = np.arange(E) - starts[rid]
    pos = off[ge] + (le - (ge << 5)) * W_grp[ge] + slot

    x0s = np.zeros((N_CORES, P, TOT), np.float32)
    x1s = np.zeros((N_CORES, P, TOT), np.float32)
    dgs = np.ones((N_CORES, P, TOT), np.float32)
    core_masks = []
    for c in range(N_CORES):
        m = ce == c
        core_masks.append(m)
        x0s[c][pe[m], pos[m]] = x[row[m], 0]
        x1s[c][pe[m], pos[m]] = x[row[m], 1]
        dgs[c][pe[m], pos[m]] = deg[row[m]]

    # ---- node tables ----
    pgrid = np.arange(P)[:, None]
    lgrid = np.arange(NSL)[None, :]
    x0t = np.zeros((N_CORES, P, NSL), np.float32)
    x1t = np.zeros((N_CORES, P, NSL), np.float32)
    dgt = np.ones((N_CORES, P, NSL), np.float32)
    nodes_c = []
    valid_c = []
    for c in range(N_CORES):
        ranks = (lgrid * N_CORES + c) * P + pgrid          # [P, NSL]
        valid = ranks < N
        nodes = order[np.minimum(ranks, N - 1)]
        nodes_c.append(nodes); valid_c.append(valid)
        x0t[c] = np.where(valid, x[nodes, 0], 0.0)
        x1t[c] = np.where(valid, x[nodes, 1], 0.0)
        dgt[c] = np.where(valid, deg[nodes].astype(np.float32), 1.0)

    wb = np.zeros((P, 65), np.float32)
    wb[:, 0:16] = W1[0]; wb[:, 16:32] = W1[1]
    wb[:, 32:48] = b1
    wb[:, 48:64] = W2[:, 0]
    wb[:, 64] = b2[0]

    in_maps_A = [{"x0s": x0s[c], "x1s": x1s[c], "dgs": dgs[c],
                  "x0t": x0t[c], "x1t": x1t[c], "dgt": dgt[c], "wb": wb}
                 for c in range(N_CORES)]
    resA = run_bass_kernel_spmd(ncA, in_maps_A, core_ids=list(range(N_CORES)),
                                trace=TRACE)

    # ---- sigma table, host re-shard into slots ----
    sigma = np.zeros(N, np.float32)
    for c in range(N_CORES):
        v = valid_c[c]
        sigma[nodes_c[c][v]] = resA.results[c]["sOut"][v]

    sgs = np.zeros((N_CORES, P, TOT), np.float32)
    st = np.zeros((N_CORES, P, NSL), np.float32)
    for c in range(N_CORES):
        m = core_masks[c]
        sgs[c][pe[m], pos[m]] = sigma[row[m]]
        st[c] = np.where(valid_c[c], sigma[nodes_c[c]], 0.0)

    in_maps_B = [{"sgs": sgs[c], "dgtB": dgt[c], "stB": st[c], "wbB": wb}
                 for c in range(N_CORES)]
    resB = run_bass_kernel_spmd(ncB, in_maps_B, core_ids=list(range(N_CORES)),
                                trace=TRACE)

    out = np.zeros(N, np.float32)
    for c in range(N_CORES):
        v = valid_c[c]
        out[nodes_c[c][v]] = resB.results[c]["out"][v]

    kernel.last_exec_ns = (getattr(resA, "exec_time_ns", None) or 0) + \
                          (getattr(resB, "exec_time_ns", None) or 0)
    return out[:, None]



# revision 9
# speedup vs baseline: 1.5926x; 1.5926x over previous
"""2-layer GCN (GCNConv -> ReLU -> GCNConv) on 8 TRN2 NeuronCores.

Strategy (sliced-ELLPACK, node sharding, bf16 slots):
  GCN algebra: out = D^-1/2 (A+I) D^-1/2 (relu(D^-1/2 (A+I) D^-1/2 x W1 + b1)) W2 + b2.
  Normalization is separable (norm_e = dinv[row]*dinv[col]) and aggregation
  linear, so each layer is: per-node scale -> unweighted neighbor-sum ->
  per-node scale -> dense matmul. Self-loop contributions are dense adds.

  Host (pure index layout, no model math): sort nodes by in-degree, pack each
  node's incoming edges into padded slot rows (sliced ELLPACK, slices of 128
  nodes, DP-chosen groups of slices sharing a width). Slots carry the *input*
  features x[row] (bf16) and the integer structural count deg[row] (bf16,
  exact). Device computes all FP model math: rsqrt on the scalar engine,
  per-slot scaling + segmented reductions on DVE (bf16 2x mode), the
  2->16->1 MLP as a scalar_tensor_tensor chain with the per-j feature scale
  folded into the scalar-engine relu via a device-computed b/a ratio.

  Two launches: A computes layer 1 + q = dinv * (h @ W2); host re-shards q
  into the same slot layout (index gather only); B reduces q-slots and
  finishes layer 2. Nodes sharded round-robin by 128-node slice; edge slots
  live with their target node.
"""
import numpy as np
import ml_dtypes

BF16 = ml_dtypes.bfloat16
P = 128
N_CORES = 8
NSL = 256            # local slices per core -> 2048 global slices
NRANKS = 2048 * P    # padded rank space (262144 >= 250000)
LAM = 60             # DP penalty (columns) per extra width-group
N_CHUNKS = 3         # slot-array DMA chunks

TRACE = False
_cache = {}


def _install_ntff_shim():
    import contextlib, ctypes, sys, types
    if "antenv.axon_hooks" in sys.modules:
        return
    try:
        lib = ctypes.CDLL("/opt/axon/libaxon_pjrt.so")
        if not hasattr(lib, "axon_start_nrt_profile"):
            return
        lib.axon_start_nrt_profile.argtypes = [ctypes.POINTER(ctypes.c_int64), ctypes.c_size_t]
        lib.axon_start_nrt_profile.restype = ctypes.c_int64
        lib.axon_stop_nrt_profile.argtypes = [ctypes.c_char_p]
        lib.axon_stop_nrt_profile.restype = ctypes.c_int64
    except OSError:
        return

    @contextlib.contextmanager
    def _hook(output_dir, device_ids):
        import jax
        jax.devices()
        if device_ids:
            ids = (ctypes.c_int64 * len(device_ids))(*device_ids)
            rc = lib.axon_start_nrt_profile(ids, len(device_ids))
        else:
            rc = lib.axon_start_nrt_profile(None, 0)
        if rc != 0:
            raise RuntimeError(f"axon_start_nrt_profile rc={rc}")
        try:
            yield
        finally:
            lib.axon_stop_nrt_profile(str(output_dir).encode())

    mod = types.ModuleType("antenv.axon_hooks")
    mod.get_axon_ntff_profile_hook = lambda: _hook
    mod.set_axon_ntff_profile_hook = lambda h: None
    sys.modules["antenv.axon_hooks"] = mod


def _plan_groups(W_l):
    """DP partition of local slices into contiguous width-groups."""
    nsl = len(W_l)
    INF = 1 << 60
    best = np.full(nsl + 1, INF, np.int64)
    best[0] = 0
    ch = np.zeros(nsl + 1, np.int64)
    for e in range(1, nsl + 1):
        s = np.arange(e)
        c = best[:e] + (e - s) * W_l[s] + LAM
        i = int(np.argmin(c))
        best[e] = c[i]
        ch[e] = i
    bnds = []
    e = nsl
    while e > 0:
        s = int(ch[e])
        bnds.append((s, e))
        e = s
    bnds.reverse()
    groups = []
    off = 0
    for s, e in bnds:
        w = int(W_l[s])
        groups.append((s, e, w, off))   # (l0, l1, width, column offset in dgs)
        off += (e - s) * w
    return groups, off                  # TOT = off


def _chunk_groups(groups):
    """Split groups into N_CHUNKS column-balanced DMA chunks."""
    tot = sum((e - s) * w for s, e, w, _ in groups)
    chunks = []
    cur = []
    acc = 0
    target = tot / N_CHUNKS
    for g in groups:
        s, e, w, _ = g
        cur.append(g)
        acc += (e - s) * w
        if acc >= target * (len(chunks) + 1) and len(chunks) < N_CHUNKS - 1:
            chunks.append(cur)
            cur = []
    if cur:
        chunks.append(cur)
    return [c for c in chunks if c]


def _build_programs(groups, TOT):
    import concourse.bass as bass
    import concourse.bacc as bacc
    import concourse.tile as tile
    import concourse.mybir as mybir

    f32 = mybir.dt.float32
    bf16 = mybir.dt.bfloat16
    AF = mybir.ActivationFunctionType
    ALU = mybir.AluOpType
    X = mybir.AxisListType.X
    chunks = _chunk_groups(groups)

    # ---------------- program A ----------------
    ncA = bacc.Bacc("TRN2", target_bir_lowering=False, debug=False, num_devices=N_CORES)
    x01s = ncA.dram_tensor("x01s", [P, 2 * TOT], bf16, kind="ExternalInput")
    dgs = ncA.dram_tensor("dgs", [P, TOT], bf16, kind="ExternalInput")
    tb = ncA.dram_tensor("tb", [P, 768], bf16, kind="ExternalInput")    # xt01 | dgt
    wb = ncA.dram_tensor("wb", [P, 65], f32, kind="ExternalInput")
    sOut = ncA.dram_tensor("sOut", [P, NSL], f32, kind="ExternalOutput")

    with tile.TileContext(ncA) as tc:
        with tc.tile_pool(name="slots", bufs=2) as pool, \
             tc.tile_pool(name="persist", bufs=1) as pp:
            tbt = pp.tile([P, 768], bf16)
            ncA.sync.dma_start(tbt[:], tb.ap())
            wbt = pp.tile([P, 65], f32)
            ncA.sync.dma_start(wbt[:], wb.ap())
            agg = pp.tile([P, 512], f32)

            # node-table prep: dinv, dinv^2, weight ratios
            dinv = pp.tile([P, NSL], bf16)
            ncA.scalar.activation(dinv[:], tbt[:, 512:768], AF.Abs_reciprocal_sqrt)
            ta = pp.tile([P, 16], f32)
            ncA.vector.tensor_scalar(out=ta[:], in0=wbt[:, 0:16], scalar1=1e-30,
                                     scalar2=None, op0=ALU.add)
            ratio = pp.tile([P, 16], f32)
            ncA.vector.reciprocal(ratio[:], ta[:])
            ncA.vector.tensor_tensor(out=ratio[:], in0=wbt[:, 16:32], in1=ratio[:],
                                     op=ALU.mult)

            # slot chunks: load, scale by rsqrt(deg[row]), segmented-sum
            for ci, cgroups in enumerate(chunks):
                c0 = cgroups[0][3]                       # column offset of chunk
                ccols = sum((e - s) * w for s, e, w, _ in cgroups)
                xt = pool.tile([P, 2 * ccols], bf16, tag="x01")
                ncA.sync.dma_start(xt[:], x01s.ap()[:, 2 * c0:2 * (c0 + ccols)])
                dt = pool.tile([P, ccols], bf16, tag="dg")
                ncA.sync.dma_start(dt[:], dgs.ap()[:, c0:c0 + ccols])
                rt = pool.tile([P, ccols], bf16, tag="r")
                for (l0, l1, w, off) in cgroups:
                    if w == 0:
                        continue
                    sz = (l1 - l0) * w
                    o = off - c0                         # local col offset in chunk
                    ncA.scalar.activation(rt[:, o:o + sz], dt[:, o:o + sz], AF.Abs_reciprocal_sqrt)
                    ncA.vector.tensor_tensor(
                        out=xt[:, 2 * o:2 * o + sz],
                        in0=xt[:, 2 * o:2 * o + sz], in1=rt[:, o:o + sz], op=ALU.mult)
                    ncA.vector.tensor_tensor(
                        out=xt[:, 2 * o + sz:2 * o + 2 * sz],
                        in0=xt[:, 2 * o + sz:2 * o + 2 * sz], in1=rt[:, o:o + sz],
                        op=ALU.mult)
                    ncA.vector.tensor_reduce(
                        out=agg[:].rearrange("p (f n) -> p f n", f=2)[:, :, l0:l1],
                        in_=xt[:, 2 * o:2 * o + 2 * sz].rearrange(
                            "p (f n w) -> p f n w", f=2, w=w),
                        axis=X, op=ALU.add)

            # memset agg columns not covered by any group (w == 0)
            covered = np.zeros(NSL, bool)
            for (l0, l1, w, _) in groups:
                if w > 0:
                    covered[l0:l1] = True
            run = None
            for l in range(NSL + 1):
                if l < NSL and not covered[l]:
                    run = l if run is None else run
                elif run is not None:
                    for half in range(2):
                        ncA.gpsimd.memset(agg[:, half * 256 + run:half * 256 + l], 0.0)
                    run = None

            # z = dinv * (agg + dinv * x)   (both features in one [P,512] tile)
            z = pp.tile([P, 512], bf16)
            ncA.vector.tensor_tensor(
                out=z[:].rearrange("p (f n) -> p f n", f=2),
                in0=tbt[:, 0:512].rearrange("p (f n) -> p f n", f=2),
                in1=dinv[:].rearrange("p (a n) -> p a n", a=1).to_broadcast([P, 2, NSL]),
                op=ALU.mult)
            ncA.vector.tensor_tensor(out=z[:], in0=z[:], in1=agg[:], op=ALU.add)
            ncA.vector.tensor_tensor(
                out=z[:].rearrange("p (f n) -> p f n", f=2),
                in0=z[:].rearrange("p (f n) -> p f n", f=2),
                in1=dinv[:].rearrange("p (a n) -> p a n", a=1).to_broadcast([P, 2, NSL]),
                op=ALU.mult)

            # h_j = relu(a_j z0 + b_j z1 + c_j) = relu(a'_j * (z0 + ratio_j z1) + c_j)
            # acc = sum_j w2_j h_j
            z0 = z[:, 0:NSL]
            z1 = z[:, NSL:512]
            u = pp.tile([P, NSL], bf16)
            h = pp.tile([P, NSL], bf16)
            acc = pp.tile([P, NSL], f32)
            for j in range(16):
                ncA.vector.scalar_tensor_tensor(
                    out=u[:], in0=z1, scalar=ratio[:, j:j + 1], in1=z0,
                    op0=ALU.mult, op1=ALU.add)
                ncA.scalar.activation(h[:], u[:], AF.Relu,
                                      bias=wbt[:, 32 + j:33 + j],
                                      scale=ta[:, j:j + 1])
                if j == 0:
                    ncA.vector.tensor_scalar(out=acc[:], in0=h[:],
                                             scalar1=wbt[:, 48:49],
                                             scalar2=None, op0=ALU.mult)
                else:
                    ncA.vector.scalar_tensor_tensor(
                        out=acc[:], in0=h[:], scalar=wbt[:, 48 + j:49 + j],
                        in1=acc[:], op0=ALU.mult, op1=ALU.add)
            # q = dinv * (h @ W2)
            ncA.vector.tensor_tensor(out=acc[:], in0=acc[:], in1=dinv[:], op=ALU.mult)
            ncA.sync.dma_start(sOut.ap(), acc[:])
    ncA.compile()

    # ---------------- program B ----------------
    ncB = bacc.Bacc("TRN2", target_bir_lowering=False, debug=False, num_devices=N_CORES)
    sgs = ncB.dram_tensor("sgs", [P, TOT], bf16, kind="ExternalInput")
    dgtB = ncB.dram_tensor("dgtB", [P, NSL], bf16, kind="ExternalInput")
    tf = ncB.dram_tensor("tf", [P, NSL + 1], f32, kind="ExternalInput")  # qt | b2
    out = ncB.dram_tensor("out", [P, NSL], f32, kind="ExternalOutput")

    with tile.TileContext(ncB) as tc:
        with tc.tile_pool(name="slots", bufs=2) as pool, \
             tc.tile_pool(name="persist", bufs=1) as pp:
            tft = pp.tile([P, NSL + 1], f32)
            ncB.sync.dma_start(tft[:], tf.ap())
            dt = pp.tile([P, NSL], bf16)
            ncB.sync.dma_start(dt[:], dgtB.ap())
            aggS = pp.tile([P, NSL], f32)
            for ci, cgroups in enumerate(chunks):
                c0 = cgroups[0][3]
                ccols = sum((e - s) * w for s, e, w, _ in cgroups)
                st = pool.tile([P, ccols], bf16, tag="s")
                ncB.sync.dma_start(st[:], sgs.ap()[:, c0:c0 + ccols])
                for (l0, l1, w, off) in cgroups:
                    if w == 0:
                        continue
                    o = off - c0
                    ncB.vector.tensor_reduce(
                        out=aggS[:, l0:l1],
                        in_=st[:, o:o + (l1 - l0) * w].rearrange(
                            "p (n w) -> p n w", w=w),
                        axis=X, op=ALU.add)
            covered = np.zeros(NSL, bool)
            for (l0, l1, w, _) in groups:
                if w > 0:
                    covered[l0:l1] = True
            run = None
            for l in range(NSL + 1):
                if l < NSL and not covered[l]:
                    run = l if run is None else run
                elif run is not None:
                    ncB.gpsimd.memset(aggS[:, run:l], 0.0)
                    run = None
            dinv = pp.tile([P, NSL], bf16)
            ncB.scalar.activation(dinv[:], dt[:], AF.Abs_reciprocal_sqrt)
            o1 = pp.tile([P, NSL], f32)
            ncB.vector.tensor_tensor(out=o1[:], in0=aggS[:], in1=tft[:, 0:NSL],
                                     op=ALU.add)
            ncB.vector.tensor_tensor(out=o1[:], in0=o1[:], in1=dinv[:], op=ALU.mult)
            ncB.vector.tensor_scalar(out=o1[:], in0=o1[:],
                                     scalar1=tft[:, NSL:NSL + 1],
                                     scalar2=None, op0=ALU.add)
            ncB.sync.dma_start(out.ap(), o1[:])
    ncB.compile()
    return ncA, ncB


def kernel(x, edge_index, W1, b1, W2, b2, n_nodes):
    from concourse.bass_utils import run_bass_kernel_spmd

    N = int(n_nodes)
    x = np.asarray(x, dtype=np.float32)
    ei = np.asarray(edge_index)
    row = ei[0].astype(np.int64)
    col = ei[1].astype(np.int64)
    W1 = np.asarray(W1, np.float32); b1 = np.asarray(b1, np.float32)
    W2 = np.asarray(W2, np.float32); b2 = np.asarray(b2, np.float32)
    E = row.shape[0]

    # ---- host index layout (structural only) ----
    deg = np.bincount(col, minlength=N) + 1           # includes self-loop
    indeg = deg - 1
    order = np.argsort(-deg, kind="stable")           # rank -> node
    rank_of = np.empty(N, np.int64)
    rank_of[order] = np.arange(N)

    indeg_byrank = np.zeros(NRANKS, np.int64)
    indeg_byrank[:N] = indeg[order]
    W_l = indeg_byrank[np.arange(NSL) * (N_CORES * P)]   # local-slice width
    groups, TOT = _plan_groups(W_l)

    key = (TOT, tuple(g[:3] for g in groups))
    if key not in _cache:
        if TRACE:
            _install_ntff_shim()
        _cache[key] = _build_programs(groups, TOT)
    ncA, ncB = _cache[key]

    # per-group lookup tables indexed by local slice
    l2w = np.zeros(NSL, np.int64)
    l2off = np.zeros(NSL, np.int64)   # dgs column of slot (le, k=0)
    l2sz = np.zeros(NSL, np.int64)    # group block size (cols)
    l2go = np.zeros(NSL, np.int64)    # group col offset
    for (l0, l1, w, off) in groups:
        l2w[l0:l1] = w
        l2go[l0:l1] = off
        l2sz[l0:l1] = (l1 - l0) * w
        l2off[l0:l1] = off + (np.arange(l0, l1) - l0) * w

    # ---- per-edge slot placement ----
    re = rank_of[col]
    pe = re & 127
    sl = re >> 7
    ce = sl % N_CORES
    le = sl // N_CORES
    sidx = np.argsort(re, kind="stable")
    re_s = re[sidx]
    runstart = np.empty(E, bool)
    runstart[0] = True
    np.not_equal(re_s[1:], re_s[:-1], out=runstart[1:])
    starts = np.flatnonzero(runstart)
    rid = np.cumsum(runstart) - 1
    slot = np.empty(E, np.int64)
    slot[sidx] = np.arange(E) - starts[rid]
    posd = l2off[le] + slot                    # column in dgs
    posx0 = l2go[le] * 2 + (l2off[le] - l2go[le]) + slot   # column in x01s (x0)
    xsz = l2sz[le]                             # x1 column = posx0 + xsz

    x01s = np.zeros((N_CORES, P, 2 * TOT), BF16)
    dgs = np.ones((N_CORES, P, TOT), BF16)
    core_masks = []
    for c in range(N_CORES):
        m = ce == c
        core_masks.append(m)
        x01s[c][pe[m], posx0[m]] = x[row[m], 0].astype(BF16)
        x01s[c][pe[m], posx0[m] + xsz[m]] = x[row[m], 1].astype(BF16)
        dgs[c][pe[m], posd[m]] = deg[row[m]].astype(BF16)

    # ---- node tables ----
    pgrid = np.arange(P)[:, None]
    lgrid = np.arange(NSL)[None, :]
    tbs = np.zeros((N_CORES, P, 768), BF16)
    nodes_c = []
    valid_c = []
    for c in range(N_CORES):
        ranks = (lgrid * N_CORES + c) * P + pgrid          # [P, NSL]
        valid = ranks < N
        nodes = order[np.minimum(ranks, N - 1)]
        nodes_c.append(nodes); valid_c.append(valid)
        tbs[c, :, 0:256] = np.where(valid, x[nodes, 0], 0.0).astype(BF16)
        tbs[c, :, 256:512] = np.where(valid, x[nodes, 1], 0.0).astype(BF16)
        tbs[c, :, 512:768] = np.where(valid, deg[nodes].astype(np.float32),
                                      1.0).astype(BF16)

    wb = np.zeros((P, 65), np.float32)
    wb[:, 0:16] = W1[0]; wb[:, 16:32] = W1[1]
    wb[:, 32:48] = b1
    wb[:, 48:64] = W2[:, 0]
    wb[:, 64] = b2[0]

    in_maps_A = [{"x01s": x01s[c], "dgs": dgs[c], "tb": tbs[c], "wb": wb}
                 for c in range(N_CORES)]
    resA = run_bass_kernel_spmd(ncA, in_maps_A, core_ids=list(range(N_CORES)),
                                trace=TRACE)

    # ---- q table, host re-shard into slots ----
    q = np.zeros(N, np.float32)
    for c in range(N_CORES):
        v = valid_c[c]
        q[nodes_c[c][v]] = resA.results[c]["sOut"][v]
    kernel._dbg = {"q": q, "resA": resA}

    sgs = np.zeros((N_CORES, P, TOT), BF16)
    tfs = np.zeros((N_CORES, P, NSL + 1), np.float32)
    qrow = q[row].astype(BF16)
    for c in range(N_CORES):
        m = core_masks[c]
        sgs[c][pe[m], posd[m]] = qrow[m]
        tfs[c, :, 0:NSL] = np.where(valid_c[c], q[nodes_c[c]], 0.0)
        tfs[c, :, NSL] = b2[0]

    in_maps_B = [{"sgs": sgs[c], "dgtB": tbs[c, :, 512:768], "tf": tfs[c]}
                 for c in range(N_CORES)]
    resB = run_bass_kernel_spmd(ncB, in_maps_B, core_ids=list(range(N_CORES)),
                                trace=TRACE)

    outv = np.zeros(N, np.float32)
    for c in range(N_CORES):
        v = valid_c[c]
        outv[nodes_c[c][v]] = resB.results[c]["out"][v]
    kernel._dbg.update({"sgs": sgs, "tfs": tfs, "resB": resB,
                        "nodes_c": nodes_c, "valid_c": valid_c,
                        "groups": groups, "TOT": TOT})

    kernel.last_exec_ns = (getattr(resA, "exec_time_ns", None) or 0) + \
                          (getattr(resB, "exec_time_ns", None) or 0)
    return outv[:, None]


# revision 11
# speedup vs baseline: 1.6476x; 1.0346x over previous
"""2-layer GCN (GCNConv -> ReLU -> GCNConv) on 8 TRN2 NeuronCores.

Strategy (sliced-ELLPACK, node sharding, bf16 slots):
  GCN algebra: out = D^-1/2 (A+I) D^-1/2 (relu(D^-1/2 (A+I) D^-1/2 x W1 + b1)) W2 + b2.
  Normalization is separable (norm_e = dinv[row]*dinv[col]) and aggregation
  linear, so each layer is: per-node scale -> unweighted neighbor-sum ->
  per-node scale -> dense matmul. Self-loop contributions are dense adds.

  Host (pure index layout, no model math): sort nodes by in-degree, pack each
  node's incoming edges into padded slot rows (sliced ELLPACK, slices of 128
  nodes, DP-chosen groups of slices sharing a width). Slots carry the *input*
  features x[row] (bf16) and the integer structural count deg[row] (bf16,
  exact). Device computes all FP model math: rsqrt on the scalar engine,
  per-slot scaling + segmented reductions on DVE (bf16 2x mode), the
  2->16->1 MLP as a scalar_tensor_tensor chain with the per-j feature scale
  folded into the scalar-engine relu via a device-computed b/a ratio.

  Two launches: A computes layer 1 + q = dinv * (h @ W2); host re-shards q
  into the same slot layout (index gather only); B reduces q-slots and
  finishes layer 2. Nodes sharded round-robin by 128-node slice; edge slots
  live with their target node.
"""
import numpy as np
import ml_dtypes

BF16 = ml_dtypes.bfloat16
P = 128
N_CORES = 8
NSL = 256            # local slices per core -> 2048 global slices
NRANKS = 2048 * P    # padded rank space (262144 >= 250000)
LAM = 60             # DP penalty (columns) per extra width-group
N_CHUNKS = 3         # slot-array DMA chunks

TRACE = False
_cache = {}


def _install_ntff_shim():
    import contextlib, ctypes, sys, types
    if "antenv.axon_hooks" in sys.modules:
        return
    try:
        lib = ctypes.CDLL("/opt/axon/libaxon_pjrt.so")
        if not hasattr(lib, "axon_start_nrt_profile"):
            return
        lib.axon_start_nrt_profile.argtypes = [ctypes.POINTER(ctypes.c_int64), ctypes.c_size_t]
        lib.axon_start_nrt_profile.restype = ctypes.c_int64
        lib.axon_stop_nrt_profile.argtypes = [ctypes.c_char_p]
        lib.axon_stop_nrt_profile.restype = ctypes.c_int64
    except OSError:
        return

    @contextlib.contextmanager
    def _hook(output_dir, device_ids):
        import jax
        jax.devices()
        if device_ids:
            ids = (ctypes.c_int64 * len(device_ids))(*device_ids)
            rc = lib.axon_start_nrt_profile(ids, len(device_ids))
        else:
            rc = lib.axon_start_nrt_profile(None, 0)
        if rc != 0:
            raise RuntimeError(f"axon_start_nrt_profile rc={rc}")
        try:
            yield
        finally:
            lib.axon_stop_nrt_profile(str(output_dir).encode())

    mod = types.ModuleType("antenv.axon_hooks")
    mod.get_axon_ntff_profile_hook = lambda: _hook
    mod.set_axon_ntff_profile_hook = lambda h: None
    sys.modules["antenv.axon_hooks"] = mod


def _plan_groups(W_l):
    """DP partition of local slices into contiguous width-groups.
    Boundaries restricted to even slice indices so every group block has an
    even column count (keeps bf16 sub-blocks 4B-aligned for DVE 2x mode)."""
    nsl = len(W_l)
    INF = 1 << 60
    best = np.full(nsl + 1, INF, np.int64)
    best[0] = 0
    ch = np.zeros(nsl + 1, np.int64)
    for e in range(2, nsl + 1, 2):
        s = np.arange(0, e, 2)
        c = best[s] + (e - s) * W_l[s] + LAM
        i = int(np.argmin(c))
        best[e] = c[i]
        ch[e] = 2 * i
    bnds = []
    e = nsl
    while e > 0:
        s = int(ch[e])
        bnds.append((s, e))
        e = s
    bnds.reverse()
    groups = []
    off = 0
    for s, e in bnds:
        w = int(W_l[s])
        groups.append((s, e, w, off))   # (l0, l1, width, column offset in dgs)
        off += (e - s) * w
    return groups, off                  # TOT = off


def _chunk_groups(groups):
    """Split groups into N_CHUNKS column-balanced DMA chunks."""
    tot = sum((e - s) * w for s, e, w, _ in groups)
    chunks = []
    cur = []
    acc = 0
    target = tot / N_CHUNKS
    for g in groups:
        s, e, w, _ = g
        cur.append(g)
        acc += (e - s) * w
        if acc >= target * (len(chunks) + 1) and len(chunks) < N_CHUNKS - 1:
            chunks.append(cur)
            cur = []
    if cur:
        chunks.append(cur)
    return [c for c in chunks if c]


def _build_programs(groups, TOT):
    import concourse.bass as bass
    import concourse.bacc as bacc
    import concourse.tile as tile
    import concourse.mybir as mybir

    f32 = mybir.dt.float32
    bf16 = mybir.dt.bfloat16
    AF = mybir.ActivationFunctionType
    ALU = mybir.AluOpType
    X = mybir.AxisListType.X
    chunks = _chunk_groups(groups)

    # ---------------- program A ----------------
    ncA = bacc.Bacc("TRN2", target_bir_lowering=False, debug=False, num_devices=N_CORES)
    x01s = ncA.dram_tensor("x01s", [P, 2 * TOT], bf16, kind="ExternalInput")
    dgs = ncA.dram_tensor("dgs", [P, TOT], bf16, kind="ExternalInput")
    tb = ncA.dram_tensor("tb", [P, 768], bf16, kind="ExternalInput")    # xt01 | dgt
    wb = ncA.dram_tensor("wb", [P, 65], f32, kind="ExternalInput")
    sOut = ncA.dram_tensor("sOut", [P, NSL], f32, kind="ExternalOutput")

    with tile.TileContext(ncA) as tc:
        with tc.tile_pool(name="slots", bufs=2) as pool, \
             tc.tile_pool(name="persist", bufs=1) as pp:
            tbt = pp.tile([P, 768], bf16)
            ncA.scalar.dma_start(tbt[:], tb.ap())
            wbt = pp.tile([P, 65], f32)
            ncA.scalar.dma_start(wbt[:], wb.ap())
            agg = pp.tile([P, 512], f32)

            # node-table prep: dinv, dinv^2, weight ratios
            dinv = pp.tile([P, NSL], bf16)
            ncA.scalar.activation(dinv[:], tbt[:, 512:768], AF.Abs_reciprocal_sqrt)
            ta = pp.tile([P, 16], f32)
            ncA.vector.tensor_scalar(out=ta[:], in0=wbt[:, 0:16], scalar1=1e-30,
                                     scalar2=None, op0=ALU.add)
            ratio = pp.tile([P, 16], f32)
            ncA.vector.reciprocal(ratio[:], ta[:])
            ncA.vector.tensor_tensor(out=ratio[:], in0=wbt[:, 16:32], in1=ratio[:],
                                     op=ALU.mult)

            # slot chunks: load, scale by rsqrt(deg[row]), segmented-sum
            for ci, cgroups in enumerate(chunks):
                c0 = cgroups[0][3]                       # column offset of chunk
                ccols = sum((e - s) * w for s, e, w, _ in cgroups)
                xt = pool.tile([P, 2 * ccols], bf16, tag="x01")
                ncA.sync.dma_start(xt[:], x01s.ap()[:, 2 * c0:2 * (c0 + ccols)])
                dt = pool.tile([P, ccols], bf16, tag="dg")
                ncA.scalar.dma_start(dt[:], dgs.ap()[:, c0:c0 + ccols])
                rt = pool.tile([P, ccols], bf16, tag="r")
                for (l0, l1, w, off) in cgroups:
                    if w == 0:
                        continue
                    sz = (l1 - l0) * w
                    o = off - c0                         # local col offset in chunk
                    ncA.scalar.activation(rt[:, o:o + sz], dt[:, o:o + sz], AF.Abs_reciprocal_sqrt)
                    ncA.vector.tensor_tensor(
                        out=xt[:, 2 * o:2 * o + sz],
                        in0=xt[:, 2 * o:2 * o + sz], in1=rt[:, o:o + sz], op=ALU.mult)
                    ncA.vector.tensor_tensor(
                        out=xt[:, 2 * o + sz:2 * o + 2 * sz],
                        in0=xt[:, 2 * o + sz:2 * o + 2 * sz], in1=rt[:, o:o + sz],
                        op=ALU.mult)
                    ncA.vector.tensor_reduce(
                        out=agg[:].rearrange("p (f n) -> p f n", f=2)[:, :, l0:l1],
                        in_=xt[:, 2 * o:2 * o + 2 * sz].rearrange(
                            "p (f n w) -> p f n w", f=2, w=w),
                        axis=X, op=ALU.add)

            # memset agg columns not covered by any group (w == 0)
            covered = np.zeros(NSL, bool)
            for (l0, l1, w, _) in groups:
                if w > 0:
                    covered[l0:l1] = True
            run = None
            for l in range(NSL + 1):
                if l < NSL and not covered[l]:
                    run = l if run is None else run
                elif run is not None:
                    for half in range(2):
                        ncA.gpsimd.memset(agg[:, half * 256 + run:half * 256 + l], 0.0)
                    run = None

            # z = dinv * (agg + dinv * x)   (both features in one [P,512] tile)
            z = pp.tile([P, 512], bf16)
            ncA.vector.tensor_tensor(
                out=z[:].rearrange("p (f n) -> p f n", f=2),
                in0=tbt[:, 0:512].rearrange("p (f n) -> p f n", f=2),
                in1=dinv[:].rearrange("p (a n) -> p a n", a=1).to_broadcast([P, 2, NSL]),
                op=ALU.mult)
            ncA.vector.tensor_tensor(out=z[:], in0=z[:], in1=agg[:], op=ALU.add)
            ncA.vector.tensor_tensor(
                out=z[:].rearrange("p (f n) -> p f n", f=2),
                in0=z[:].rearrange("p (f n) -> p f n", f=2),
                in1=dinv[:].rearrange("p (a n) -> p a n", a=1).to_broadcast([P, 2, NSL]),
                op=ALU.mult)

            # h_j = relu(a_j z0 + b_j z1 + c_j) = relu(a'_j * (z0 + ratio_j z1) + c_j)
            # acc = sum_j w2_j h_j
            z0 = z[:, 0:NSL]
            z1 = z[:, NSL:512]
            u = pp.tile([P, NSL], bf16)
            h = pp.tile([P, NSL], bf16)
            acc = pp.tile([P, NSL], bf16)
            q = pp.tile([P, NSL], f32)
            for j in range(16):
                ncA.vector.scalar_tensor_tensor(
                    out=u[:], in0=z1, scalar=ratio[:, j:j + 1], in1=z0,
                    op0=ALU.mult, op1=ALU.add)
                ncA.scalar.activation(h[:], u[:], AF.Relu,
                                      bias=wbt[:, 32 + j:33 + j],
                                      scale=ta[:, j:j + 1])
                if j == 0:
                    ncA.vector.tensor_scalar(out=acc[:], in0=h[:],
                                             scalar1=wbt[:, 48:49],
                                             scalar2=None, op0=ALU.mult)
                else:
                    ncA.vector.scalar_tensor_tensor(
                        out=acc[:], in0=h[:], scalar=wbt[:, 48 + j:49 + j],
                        in1=acc[:], op0=ALU.mult, op1=ALU.add)
            # q = dinv * (h @ W2)
            ncA.vector.tensor_tensor(out=q[:], in0=acc[:], in1=dinv[:], op=ALU.mult)
            ncA.sync.dma_start(sOut.ap(), q[:])
    ncA.compile()

    # ---------------- program B ----------------
    ncB = bacc.Bacc("TRN2", target_bir_lowering=False, debug=False, num_devices=N_CORES)
    sgs = ncB.dram_tensor("sgs", [P, TOT], bf16, kind="ExternalInput")
    dgtB = ncB.dram_tensor("dgtB", [P, NSL], bf16, kind="ExternalInput")
    tf = ncB.dram_tensor("tf", [P, NSL + 1], f32, kind="ExternalInput")  # qt | b2
    out = ncB.dram_tensor("out", [P, NSL], f32, kind="ExternalOutput")

    with tile.TileContext(ncB) as tc:
        with tc.tile_pool(name="slots", bufs=2) as pool, \
             tc.tile_pool(name="persist", bufs=1) as pp:
            tft = pp.tile([P, NSL + 1], f32)
            ncB.scalar.dma_start(tft[:], tf.ap())
            dt = pp.tile([P, NSL], bf16)
            ncB.scalar.dma_start(dt[:], dgtB.ap())
            aggS = pp.tile([P, NSL], f32)
            for ci, cgroups in enumerate(chunks):
                c0 = cgroups[0][3]
                ccols = sum((e - s) * w for s, e, w, _ in cgroups)
                st = pool.tile([P, ccols], bf16, tag="s")
                ncB.sync.dma_start(st[:], sgs.ap()[:, c0:c0 + ccols])
                for (l0, l1, w, off) in cgroups:
                    if w == 0:
                        continue
                    o = off - c0
                    ncB.vector.tensor_reduce(
                        out=aggS[:, l0:l1],
                        in_=st[:, o:o + (l1 - l0) * w].rearrange(
                            "p (n w) -> p n w", w=w),
                        axis=X, op=ALU.add)
            covered = np.zeros(NSL, bool)
            for (l0, l1, w, _) in groups:
                if w > 0:
                    covered[l0:l1] = True
            run = None
            for l in range(NSL + 1):
                if l < NSL and not covered[l]:
                    run = l if run is None else run
                elif run is not None:
                    ncB.gpsimd.memset(aggS[:, run:l], 0.0)
                    run = None
            dinv = pp.tile([P, NSL], bf16)
            ncB.scalar.activation(dinv[:], dt[:], AF.Abs_reciprocal_sqrt)
            o1 = pp.tile([P, NSL], f32)
            ncB.vector.tensor_tensor(out=o1[:], in0=aggS[:], in1=tft[:, 0:NSL],
                                     op=ALU.add)
            ncB.vector.tensor_tensor(out=o1[:], in0=o1[:], in1=dinv[:], op=ALU.mult)
            ncB.vector.tensor_scalar(out=o1[:], in0=o1[:],
                                     scalar1=tft[:, NSL:NSL + 1],
                                     scalar2=None, op0=ALU.add)
            ncB.sync.dma_start(out.ap(), o1[:])
    ncB.compile()
    return ncA, ncB


def kernel(x, edge_index, W1, b1, W2, b2, n_nodes):
    from concourse.bass_utils import run_bass_kernel_spmd

    N = int(n_nodes)
    x = np.asarray(x, dtype=np.float32)
    ei = np.asarray(edge_index)
    row = ei[0].astype(np.int64)
    col = ei[1].astype(np.int64)
    W1 = np.asarray(W1, np.float32); b1 = np.asarray(b1, np.float32)
    W2 = np.asarray(W2, np.float32); b2 = np.asarray(b2, np.float32)
    E = row.shape[0]

    # ---- host index layout (structural only) ----
    deg = np.bincount(col, minlength=N) + 1           # includes self-loop
    indeg = deg - 1
    order = np.argsort(-deg, kind="stable")           # rank -> node
    rank_of = np.empty(N, np.int64)
    rank_of[order] = np.arange(N)

    indeg_byrank = np.zeros(NRANKS, np.int64)
    indeg_byrank[:N] = indeg[order]
    W_l = indeg_byrank[np.arange(NSL) * (N_CORES * P)]   # local-slice width
    groups, TOT = _plan_groups(W_l)

    key = (TOT, tuple(g[:3] for g in groups))
    if key not in _cache:
        if TRACE:
            _install_ntff_shim()
        _cache[key] = _build_programs(groups, TOT)
    ncA, ncB = _cache[key]

    # per-group lookup tables indexed by local slice
    l2w = np.zeros(NSL, np.int64)
    l2off = np.zeros(NSL, np.int64)   # dgs column of slot (le, k=0)
    l2sz = np.zeros(NSL, np.int64)    # group block size (cols)
    l2go = np.zeros(NSL, np.int64)    # group col offset
    for (l0, l1, w, off) in groups:
        l2w[l0:l1] = w
        l2go[l0:l1] = off
        l2sz[l0:l1] = (l1 - l0) * w
        l2off[l0:l1] = off + (np.arange(l0, l1) - l0) * w

    # ---- per-edge slot placement ----
    re = rank_of[col]
    pe = re & 127
    sl = re >> 7
    ce = sl % N_CORES
    le = sl // N_CORES
    sidx = np.argsort(re, kind="stable")
    re_s = re[sidx]
    runstart = np.empty(E, bool)
    runstart[0] = True
    np.not_equal(re_s[1:], re_s[:-1], out=runstart[1:])
    starts = np.flatnonzero(runstart)
    rid = np.cumsum(runstart) - 1
    slot = np.empty(E, np.int64)
    slot[sidx] = np.arange(E) - starts[rid]
    posd = l2off[le] + slot                    # column in dgs
    posx0 = l2go[le] * 2 + (l2off[le] - l2go[le]) + slot   # column in x01s (x0)
    xsz = l2sz[le]                             # x1 column = posx0 + xsz

    x01s = np.zeros((N_CORES, P, 2 * TOT), BF16)
    dgs = np.ones((N_CORES, P, TOT), BF16)
    core_masks = []
    for c in range(N_CORES):
        m = ce == c
        core_masks.append(m)
        x01s[c][pe[m], posx0[m]] = x[row[m], 0].astype(BF16)
        x01s[c][pe[m], posx0[m] + xsz[m]] = x[row[m], 1].astype(BF16)
        dgs[c][pe[m], posd[m]] = deg[row[m]].astype(BF16)

    # ---- node tables ----
    pgrid = np.arange(P)[:, None]
    lgrid = np.arange(NSL)[None, :]
    tbs = np.zeros((N_CORES, P, 768), BF16)
    nodes_c = []
    valid_c = []
    for c in range(N_CORES):
        ranks = (lgrid * N_CORES + c) * P + pgrid          # [P, NSL]
        valid = ranks < N
        nodes = order[np.minimum(ranks, N - 1)]
        nodes_c.append(nodes); valid_c.append(valid)
        tbs[c, :, 0:256] = np.where(valid, x[nodes, 0], 0.0).astype(BF16)
        tbs[c, :, 256:512] = np.where(valid, x[nodes, 1], 0.0).astype(BF16)
        tbs[c, :, 512:768] = np.where(valid, deg[nodes].astype(np.float32),
                                      1.0).astype(BF16)

    wb = np.zeros((P, 65), np.float32)
    wb[:, 0:16] = W1[0]; wb[:, 16:32] = W1[1]
    wb[:, 32:48] = b1
    wb[:, 48:64] = W2[:, 0]
    wb[:, 64] = b2[0]

    in_maps_A = [{"x01s": x01s[c], "dgs": dgs[c], "tb": tbs[c], "wb": wb}
                 for c in range(N_CORES)]
    resA = run_bass_kernel_spmd(ncA, in_maps_A, core_ids=list(range(N_CORES)),
                                trace=TRACE)

    # ---- q table, host re-shard into slots ----
    q = np.zeros(N, np.float32)
    for c in range(N_CORES):
        v = valid_c[c]
        q[nodes_c[c][v]] = resA.results[c]["sOut"][v]
    kernel._dbg = {"q": q, "resA": resA}

    sgs = np.zeros((N_CORES, P, TOT), BF16)
    tfs = np.zeros((N_CORES, P, NSL + 1), np.float32)
    qrow = q[row].astype(BF16)
    for c in range(N_CORES):
        m = core_masks[c]
        sgs[c][pe[m], posd[m]] = qrow[m]
        tfs[c, :, 0:NSL] = np.where(valid_c[c], q[nodes_c[c]], 0.0)
        tfs[c, :, NSL] = b2[0]

    in_maps_B = [{"sgs": sgs[c], "dgtB": tbs[c, :, 512:768], "tf": tfs[c]}
                 for c in range(N_CORES)]
    resB = run_bass_kernel_spmd(ncB, in_maps_B, core_ids=list(range(N_CORES)),
                                trace=TRACE)

    outv = np.zeros(N, np.float32)
    for c in range(N_CORES):
        v = valid_c[c]
        outv[nodes_c[c][v]] = resB.results[c]["out"][v]
    kernel._dbg.update({"sgs": sgs, "tfs": tfs, "resB": resB,
                        "nodes_c": nodes_c, "valid_c": valid_c,
                        "groups": groups, "TOT": TOT})

    kernel.last_exec_ns = (getattr(resA, "exec_time_ns", None) or 0) + \
                          (getattr(resB, "exec_time_ns", None) or 0)
    return outv[:, None]


# revision 12
# speedup vs baseline: 1.9521x; 1.1848x over previous
"""2-layer GCN (GCNConv -> ReLU -> GCNConv) on 8 TRN2 NeuronCores.

Strategy (sliced-ELLPACK, node sharding, bf16 slots):
  GCN algebra: out = D^-1/2 (A+I) D^-1/2 (relu(D^-1/2 (A+I) D^-1/2 x W1 + b1)) W2 + b2.
  Normalization is separable (norm_e = dinv[row]*dinv[col]) and aggregation
  linear, so each layer is: per-node scale -> unweighted neighbor-sum ->
  per-node scale -> dense matmul. Self-loop contributions are dense adds.

  Host (pure index layout, no model math): sort nodes by in-degree, pack each
  node's incoming edges into padded slot rows (sliced ELLPACK, slices of 128
  nodes, DP-chosen groups of slices sharing a width). Slots carry the *input*
  features x[row] (bf16) and the integer structural count deg[row] (bf16,
  exact). Device computes all FP model math: rsqrt on the scalar engine,
  per-slot scaling + segmented reductions on DVE (bf16 2x mode), the
  2->16->1 MLP as a scalar_tensor_tensor chain with the per-j feature scale
  folded into the scalar-engine relu via a device-computed b/a ratio.

  Two launches: A computes layer 1 + q = dinv * (h @ W2); host re-shards q
  into the same slot layout (index gather only); B reduces q-slots and
  finishes layer 2. Nodes sharded round-robin by 128-node slice; edge slots
  live with their target node.
"""
import numpy as np
import ml_dtypes

BF16 = ml_dtypes.bfloat16
P = 128
N_CORES = 8
NSL = 256            # local slices per core -> 2048 global slices
NRANKS = 2048 * P    # padded rank space (262144 >= 250000)
LAM = 60             # DP penalty (columns) per extra width-group
N_CHUNKS = 3         # slot-array DMA chunks

TRACE = False
_cache = {}


def _install_ntff_shim():
    import contextlib, ctypes, sys, types
    if "antenv.axon_hooks" in sys.modules:
        return
    try:
        lib = ctypes.CDLL("/opt/axon/libaxon_pjrt.so")
        if not hasattr(lib, "axon_start_nrt_profile"):
            return
        lib.axon_start_nrt_profile.argtypes = [ctypes.POINTER(ctypes.c_int64), ctypes.c_size_t]
        lib.axon_start_nrt_profile.restype = ctypes.c_int64
        lib.axon_stop_nrt_profile.argtypes = [ctypes.c_char_p]
        lib.axon_stop_nrt_profile.restype = ctypes.c_int64
    except OSError:
        return

    @contextlib.contextmanager
    def _hook(output_dir, device_ids):
        import jax
        jax.devices()
        if device_ids:
            ids = (ctypes.c_int64 * len(device_ids))(*device_ids)
            rc = lib.axon_start_nrt_profile(ids, len(device_ids))
        else:
            rc = lib.axon_start_nrt_profile(None, 0)
        if rc != 0:
            raise RuntimeError(f"axon_start_nrt_profile rc={rc}")
        try:
            yield
        finally:
            lib.axon_stop_nrt_profile(str(output_dir).encode())

    mod = types.ModuleType("antenv.axon_hooks")
    mod.get_axon_ntff_profile_hook = lambda: _hook
    mod.set_axon_ntff_profile_hook = lambda h: None
    sys.modules["antenv.axon_hooks"] = mod


def _plan_groups(W_l):
    """DP partition of local slices into contiguous width-groups.
    Boundaries restricted to even slice indices so every group block has an
    even column count (keeps bf16 sub-blocks 4B-aligned for DVE 2x mode)."""
    nsl = len(W_l)
    INF = 1 << 60
    best = np.full(nsl + 1, INF, np.int64)
    best[0] = 0
    ch = np.zeros(nsl + 1, np.int64)
    for e in range(2, nsl + 1, 2):
        s = np.arange(0, e, 2)
        c = best[s] + (e - s) * W_l[s] + LAM
        i = int(np.argmin(c))
        best[e] = c[i]
        ch[e] = 2 * i
    bnds = []
    e = nsl
    while e > 0:
        s = int(ch[e])
        bnds.append((s, e))
        e = s
    bnds.reverse()
    groups = []
    off = 0
    for s, e in bnds:
        w = int(W_l[s])
        groups.append((s, e, w, off))   # (l0, l1, width, column offset in dgs)
        off += (e - s) * w
    return groups, off                  # TOT = off


def _chunk_groups(groups):
    """Split groups into N_CHUNKS column-balanced DMA chunks."""
    tot = sum((e - s) * w for s, e, w, _ in groups)
    chunks = []
    cur = []
    acc = 0
    target = tot / N_CHUNKS
    for g in groups:
        s, e, w, _ = g
        cur.append(g)
        acc += (e - s) * w
        if acc >= target * (len(chunks) + 1) and len(chunks) < N_CHUNKS - 1:
            chunks.append(cur)
            cur = []
    if cur:
        chunks.append(cur)
    return [c for c in chunks if c]


def _build_programs(groups, TOT):
    import concourse.bass as bass
    import concourse.bacc as bacc
    import concourse.tile as tile
    import concourse.mybir as mybir

    f32 = mybir.dt.float32
    bf16 = mybir.dt.bfloat16
    AF = mybir.ActivationFunctionType
    ALU = mybir.AluOpType
    X = mybir.AxisListType.X
    chunks = _chunk_groups(groups)

    # ---------------- program A ----------------
    ncA = bacc.Bacc("TRN2", target_bir_lowering=False, debug=False, num_devices=N_CORES)
    x01s = ncA.dram_tensor("x01s", [P, 2 * TOT], bf16, kind="ExternalInput")
    dgs = ncA.dram_tensor("dgs", [P, TOT], bf16, kind="ExternalInput")
    tb = ncA.dram_tensor("tb", [P, 768], bf16, kind="ExternalInput")    # xt01 | dgt
    wb = ncA.dram_tensor("wb", [P, 65], f32, kind="ExternalInput")
    sOut = ncA.dram_tensor("sOut", [P, NSL], f32, kind="ExternalOutput")

    with tile.TileContext(ncA) as tc:
        with tc.tile_pool(name="slots", bufs=2) as pool, \
             tc.tile_pool(name="persist", bufs=1) as pp:
            tbt = pp.tile([P, 768], bf16)
            ncA.scalar.dma_start(tbt[:], tb.ap())
            wbt = pp.tile([P, 65], f32)
            ncA.scalar.dma_start(wbt[:], wb.ap())
            agg = pp.tile([P, 512], bf16)

            # node-table prep: dinv, dinv^2, weight ratios
            dinv = pp.tile([P, NSL], bf16)
            ncA.scalar.activation(dinv[:], tbt[:, 512:768], AF.Abs_reciprocal_sqrt)
            ta = pp.tile([P, 16], f32)
            ncA.vector.tensor_scalar(out=ta[:], in0=wbt[:, 0:16], scalar1=1e-30,
                                     scalar2=None, op0=ALU.add)
            ratio = pp.tile([P, 16], f32)
            ncA.vector.reciprocal(ratio[:], ta[:])
            ncA.vector.tensor_tensor(out=ratio[:], in0=wbt[:, 16:32], in1=ratio[:],
                                     op=ALU.mult)

            # slot chunks: load, scale by rsqrt(deg[row]), segmented-sum
            for ci, cgroups in enumerate(chunks):
                c0 = cgroups[0][3]                       # column offset of chunk
                ccols = sum((e - s) * w for s, e, w, _ in cgroups)
                xt = pool.tile([P, 2 * ccols], bf16, tag="x01")
                ncA.sync.dma_start(xt[:], x01s.ap()[:, 2 * c0:2 * (c0 + ccols)])
                dt = pool.tile([P, ccols], bf16, tag="dg")
                ncA.scalar.dma_start(dt[:], dgs.ap()[:, c0:c0 + ccols])
                rt = pool.tile([P, ccols], bf16, tag="r")
                for (l0, l1, w, off) in cgroups:
                    if w == 0:
                        continue
                    sz = (l1 - l0) * w
                    o = off - c0                         # local col offset in chunk
                    ncA.scalar.activation(rt[:, o:o + sz], dt[:, o:o + sz], AF.Abs_reciprocal_sqrt)
                    ncA.vector.tensor_tensor(
                        out=xt[:, 2 * o:2 * o + sz],
                        in0=xt[:, 2 * o:2 * o + sz], in1=rt[:, o:o + sz], op=ALU.mult)
                    ncA.vector.tensor_tensor(
                        out=xt[:, 2 * o + sz:2 * o + 2 * sz],
                        in0=xt[:, 2 * o + sz:2 * o + 2 * sz], in1=rt[:, o:o + sz],
                        op=ALU.mult)
                    with ncA.allow_low_precision(reason="bf16 agg, e2e-checked"):
                        ncA.vector.tensor_reduce(
                            out=agg[:].rearrange("p (f n) -> p f n", f=2)[:, :, l0:l1],
                            in_=xt[:, 2 * o:2 * o + 2 * sz].rearrange(
                                "p (f n w) -> p f n w", f=2, w=w),
                            axis=X, op=ALU.add)

            # memset agg columns not covered by any group (w == 0)
            covered = np.zeros(NSL, bool)
            for (l0, l1, w, _) in groups:
                if w > 0:
                    covered[l0:l1] = True
            run = None
            for l in range(NSL + 1):
                if l < NSL and not covered[l]:
                    run = l if run is None else run
                elif run is not None:
                    for half in range(2):
                        ncA.gpsimd.memset(agg[:, half * 256 + run:half * 256 + l], 0.0)
                    run = None

            # z = dinv * (agg + dinv * x)   (both features in one [P,512] tile)
            z = pp.tile([P, 512], bf16)
            ncA.vector.tensor_tensor(
                out=z[:].rearrange("p (f n) -> p f n", f=2),
                in0=tbt[:, 0:512].rearrange("p (f n) -> p f n", f=2),
                in1=dinv[:].rearrange("p (a n) -> p a n", a=1).to_broadcast([P, 2, NSL]),
                op=ALU.mult)
            ncA.vector.tensor_tensor(out=z[:], in0=z[:], in1=agg[:], op=ALU.add)
            ncA.vector.tensor_tensor(
                out=z[:].rearrange("p (f n) -> p f n", f=2),
                in0=z[:].rearrange("p (f n) -> p f n", f=2),
                in1=dinv[:].rearrange("p (a n) -> p a n", a=1).to_broadcast([P, 2, NSL]),
                op=ALU.mult)

            # h_j = relu(a_j z0 + b_j z1 + c_j) = relu(a'_j * (z0 + ratio_j z1) + c_j)
            # acc = sum_j w2_j h_j
            z0 = z[:, 0:NSL]
            z1 = z[:, NSL:512]
            # u_j for all j first (DVE runs them back to back), relus pipeline
            # on the scalar engine behind them, then two independent
            # accumulation chains so acc never stalls on a fresh relu.
            U = pp.tile([P, 16 * NSL], bf16)
            H = pp.tile([P, 16 * NSL], bf16)
            accA = pp.tile([P, NSL], f32)
            accB = pp.tile([P, NSL], f32)
            q = pp.tile([P, NSL], f32)
            for j in range(16):
                ncA.vector.scalar_tensor_tensor(
                    out=U[:, j * NSL:(j + 1) * NSL], in0=z1,
                    scalar=ratio[:, j:j + 1], in1=z0,
                    op0=ALU.mult, op1=ALU.add)
            for j in range(16):
                ncA.scalar.activation(H[:, j * NSL:(j + 1) * NSL],
                                      U[:, j * NSL:(j + 1) * NSL], AF.Relu,
                                      bias=wbt[:, 32 + j:33 + j],
                                      scale=ta[:, j:j + 1])
            for j in range(16):
                dst = accA if j % 2 == 0 else accB
                hj = H[:, j * NSL:(j + 1) * NSL]
                if j < 2:
                    ncA.vector.tensor_scalar(out=dst[:], in0=hj,
                                             scalar1=wbt[:, 48 + j:49 + j],
                                             scalar2=None, op0=ALU.mult)
                else:
                    ncA.vector.scalar_tensor_tensor(
                        out=dst[:], in0=hj, scalar=wbt[:, 48 + j:49 + j],
                        in1=dst[:], op0=ALU.mult, op1=ALU.add)
            ncA.vector.tensor_tensor(out=accA[:], in0=accA[:], in1=accB[:],
                                     op=ALU.add)
            # q = dinv * (h @ W2)
            ncA.vector.tensor_tensor(out=q[:], in0=accA[:], in1=dinv[:], op=ALU.mult)
            ncA.sync.dma_start(sOut.ap(), q[:])
    ncA.compile()

    # ---------------- program B ----------------
    ncB = bacc.Bacc("TRN2", target_bir_lowering=False, debug=False, num_devices=N_CORES)
    sgs = ncB.dram_tensor("sgs", [P, TOT], bf16, kind="ExternalInput")
    dgtB = ncB.dram_tensor("dgtB", [P, NSL], bf16, kind="ExternalInput")
    tf = ncB.dram_tensor("tf", [P, NSL + 1], f32, kind="ExternalInput")  # qt | b2
    out = ncB.dram_tensor("out", [P, NSL], f32, kind="ExternalOutput")

    with tile.TileContext(ncB) as tc:
        with tc.tile_pool(name="slots", bufs=2) as pool, \
             tc.tile_pool(name="persist", bufs=1) as pp:
            tft = pp.tile([P, NSL + 1], f32)
            ncB.scalar.dma_start(tft[:], tf.ap())
            dt = pp.tile([P, NSL], bf16)
            ncB.scalar.dma_start(dt[:], dgtB.ap())
            aggS = pp.tile([P, NSL], bf16)
            for ci, cgroups in enumerate(chunks):
                c0 = cgroups[0][3]
                ccols = sum((e - s) * w for s, e, w, _ in cgroups)
                st = pool.tile([P, ccols], bf16, tag="s")
                ncB.sync.dma_start(st[:], sgs.ap()[:, c0:c0 + ccols])
                for (l0, l1, w, off) in cgroups:
                    if w == 0:
                        continue
                    o = off - c0
                    with ncB.allow_low_precision(reason="bf16 agg, e2e-checked"):
                        ncB.vector.tensor_reduce(
                            out=aggS[:, l0:l1],
                            in_=st[:, o:o + (l1 - l0) * w].rearrange(
                                "p (n w) -> p n w", w=w),
                            axis=X, op=ALU.add)
            covered = np.zeros(NSL, bool)
            for (l0, l1, w, _) in groups:
                if w > 0:
                    covered[l0:l1] = True
            run = None
            for l in range(NSL + 1):
                if l < NSL and not covered[l]:
                    run = l if run is None else run
                elif run is not None:
                    ncB.gpsimd.memset(aggS[:, run:l], 0.0)
                    run = None
            dinv = pp.tile([P, NSL], bf16)
            ncB.scalar.activation(dinv[:], dt[:], AF.Abs_reciprocal_sqrt)
            o1 = pp.tile([P, NSL], f32)
            ncB.vector.tensor_tensor(out=o1[:], in0=aggS[:], in1=tft[:, 0:NSL],
                                     op=ALU.add)
            ncB.vector.tensor_tensor(out=o1[:], in0=o1[:], in1=dinv[:], op=ALU.mult)
            ncB.vector.tensor_scalar(out=o1[:], in0=o1[:],
                                     scalar1=tft[:, NSL:NSL + 1],
                                     scalar2=None, op0=ALU.add)
            ncB.sync.dma_start(out.ap(), o1[:])
    ncB.compile()
    return ncA, ncB


def kernel(x, edge_index, W1, b1, W2, b2, n_nodes):
    from concourse.bass_utils import run_bass_kernel_spmd

    N = int(n_nodes)
    x = np.asarray(x, dtype=np.float32)
    ei = np.asarray(edge_index)
    row = ei[0].astype(np.int64)
    col = ei[1].astype(np.int64)
    W1 = np.asarray(W1, np.float32); b1 = np.asarray(b1, np.float32)
    W2 = np.asarray(W2, np.float32); b2 = np.asarray(b2, np.float32)
    E = row.shape[0]

    # ---- host index layout (structural only) ----
    deg = np.bincount(col, minlength=N) + 1           # includes self-loop
    indeg = deg - 1
    order = np.argsort(-deg, kind="stable")           # rank -> node
    rank_of = np.empty(N, np.int64)
    rank_of[order] = np.arange(N)

    indeg_byrank = np.zeros(NRANKS, np.int64)
    indeg_byrank[:N] = indeg[order]
    W_l = indeg_byrank[np.arange(NSL) * (N_CORES * P)]   # local-slice width
    groups, TOT = _plan_groups(W_l)

    key = (TOT, tuple(g[:3] for g in groups))
    if key not in _cache:
        if TRACE:
            _install_ntff_shim()
        _cache[key] = _build_programs(groups, TOT)
    ncA, ncB = _cache[key]

    # per-group lookup tables indexed by local slice
    l2w = np.zeros(NSL, np.int64)
    l2off = np.zeros(NSL, np.int64)   # dgs column of slot (le, k=0)
    l2sz = np.zeros(NSL, np.int64)    # group block size (cols)
    l2go = np.zeros(NSL, np.int64)    # group col offset
    for (l0, l1, w, off) in groups:
        l2w[l0:l1] = w
        l2go[l0:l1] = off
        l2sz[l0:l1] = (l1 - l0) * w
        l2off[l0:l1] = off + (np.arange(l0, l1) - l0) * w

    # ---- per-edge slot placement ----
    re = rank_of[col]
    pe = re & 127
    sl = re >> 7
    ce = sl % N_CORES
    le = sl // N_CORES
    sidx = np.argsort(re, kind="stable")
    re_s = re[sidx]
    runstart = np.empty(E, bool)
    runstart[0] = True
    np.not_equal(re_s[1:], re_s[:-1], out=runstart[1:])
    starts = np.flatnonzero(runstart)
    rid = np.cumsum(runstart) - 1
    slot = np.empty(E, np.int64)
    slot[sidx] = np.arange(E) - starts[rid]
    posd = l2off[le] + slot                    # column in dgs
    posx0 = l2go[le] * 2 + (l2off[le] - l2go[le]) + slot   # column in x01s (x0)
    xsz = l2sz[le]                             # x1 column = posx0 + xsz

    x01s = np.zeros((N_CORES, P, 2 * TOT), BF16)
    dgs = np.ones((N_CORES, P, TOT), BF16)
    core_masks = []
    for c in range(N_CORES):
        m = ce == c
        core_masks.append(m)
        x01s[c][pe[m], posx0[m]] = x[row[m], 0].astype(BF16)
        x01s[c][pe[m], posx0[m] + xsz[m]] = x[row[m], 1].astype(BF16)
        dgs[c][pe[m], posd[m]] = deg[row[m]].astype(BF16)

    # ---- node tables ----
    pgrid = np.arange(P)[:, None]
    lgrid = np.arange(NSL)[None, :]
    tbs = np.zeros((N_CORES, P, 768), BF16)
    nodes_c = []
    valid_c = []
    for c in range(N_CORES):
        ranks = (lgrid * N_CORES + c) * P + pgrid          # [P, NSL]
        valid = ranks < N
        nodes = order[np.minimum(ranks, N - 1)]
        nodes_c.append(nodes); valid_c.append(valid)
        tbs[c, :, 0:256] = np.where(valid, x[nodes, 0], 0.0).astype(BF16)
        tbs[c, :, 256:512] = np.where(valid, x[nodes, 1], 0.0).astype(BF16)
        tbs[c, :, 512:768] = np.where(valid, deg[nodes].astype(np.float32),
                                      1.0).astype(BF16)

    wb = np.zeros((P, 65), np.float32)
    wb[:, 0:16] = W1[0]; wb[:, 16:32] = W1[1]
    wb[:, 32:48] = b1
    wb[:, 48:64] = W2[:, 0]
    wb[:, 64] = b2[0]

    in_maps_A = [{"x01s": x01s[c], "dgs": dgs[c], "tb": tbs[c], "wb": wb}
                 for c in range(N_CORES)]
    resA = run_bass_kernel_spmd(ncA, in_maps_A, core_ids=list(range(N_CORES)),
                                trace=TRACE)

    # ---- q table, host re-shard into slots ----
    q = np.zeros(N, np.float32)
    for c in range(N_CORES):
        v = valid_c[c]
        q[nodes_c[c][v]] = resA.results[c]["sOut"][v]
    kernel._dbg = {"q": q, "resA": resA}

    sgs = np.zeros((N_CORES, P, TOT), BF16)
    tfs = np.zeros((N_CORES, P, NSL + 1), np.float32)
    qrow = q[row].astype(BF16)
    for c in range(N_CORES):
        m = core_masks[c]
        sgs[c][pe[m], posd[m]] = qrow[m]
        tfs[c, :, 0:NSL] = np.where(valid_c[c], q[nodes_c[c]], 0.0)
        tfs[c, :, NSL] = b2[0]

    in_maps_B = [{"sgs": sgs[c], "dgtB": tbs[c, :, 512:768], "tf": tfs[c]}
                 for c in range(N_CORES)]
    resB = run_bass_kernel_spmd(ncB, in_maps_B, core_ids=list(range(N_CORES)),
                                trace=TRACE)

    outv = np.zeros(N, np.float32)
    for c in range(N_CORES):
        v = valid_c[c]
        outv[nodes_c[c][v]] = resB.results[c]["out"][v]
    kernel._dbg.update({"sgs": sgs, "tfs": tfs, "resB": resB,
                        "nodes_c": nodes_c, "valid_c": valid_c,
                        "groups": groups, "TOT": TOT})

    kernel.last_exec_ns = (getattr(resA, "exec_time_ns", None) or 0) + \
                          (getattr(resB, "exec_time_ns", None) or 0)
    return outv[:, None]
